# revision 1
# baseline (speedup 1.0000x reference)
"""Trainium2 Bass kernel for nn_CnnBasedRnn (2-layer conv-RNN).

Math: each layer computes h_t = tanh(conv3x3_stride(2,1)(concat(x_t, h_{t-1})) + b).
Because the conv input is [x_t (rows 0..63); h_{t-1} (rows 64..127)] with row
stride 2, output row i taps concat rows 2i-1..2i+1:
  rows 0..31  <- x_t only                        (bulk "A-pass")
  row  i>=32  <- h_{t-1} rows 2i-65..2i-63       (cascade regions)
Region cascade: rows 32..47 need prev-step rows <=31 (bulk), rows 48..55 need
<=47, 56..59 need <=55, 60..61 need <=59, 62 needs <=61 -- all bulk passes.
Only row 63 self-recurses (taps prev row 63); solved by fixed-point sweeps
over the whole sequence: H <- tanh(dv + W[2] (x) shift_t(H)), which is exact for
t < #sweeps and contracts by ~sum|W[2,:]| per sweep.

Layout: SBUF partitions = (img(2) x j(64)); free dim = groups of 65 slots per
timestep: slot 0 = layer_input_t[row 63], slot 1+r = h_{t-1}[row r].
Column (j) conv taps are banded 128x128 (block-diag over img) matmul weights;
row taps select slot columns via strided APs. Host pre-transposes x and
post-transposes the output, so all device DMAs are contiguous.
"""

import os
import numpy as np

B, L, D, NCORES = 16, 256, 64, 8
BS = B // NCORES          # images per core
TB = 64                   # timesteps per block
NBLK = L // TB
HB = 32                   # timesteps per half-block (A/region pass tile)


def _band(w3):
    """[64,64] banded matrix M[jin, jout] = w3[jin-jout+1] for |jin-jout|<=1."""
    M = np.zeros((D, D), np.float32)
    for dj in range(3):
        # jin - jout + 1 = dj  ->  jin = jout + dj - 1
        jout = np.arange(D)
        jin = jout + dj - 1
        m = (jin >= 0) & (jin < D)
        M[jin[m], jout[m]] = w3[dj]
    return M


def _bands_tensor(Wn):
    """[128, 6, 128]: for (l, di): block-diag over img of _band(Wn[l, di])."""
    out = np.zeros((128, 6, 128), np.float32)
    for l in range(2):
        for di in range(3):
            M = _band(Wn[l, di])
            out[0:64, l * 3 + di, 0:64] = M
            out[64:128, l * 3 + di, 64:128] = M
    return np.ascontiguousarray(out)


def _conv1d3(v, w3):
    out = (w3[1] * v).copy()
    out[..., :-1] += w3[2] * v[..., 1:]
    out[..., 1:] += w3[0] * v[..., :-1]
    return out


def _numpy_layer(xl, Wl, bl, n_iter):
    """Reference decomposition (for sweep-count estimation). xl: (b,L,D,D)."""
    nb = xl.shape[0]
    h = np.zeros((nb, L, D, D), np.float32)
    xpad = np.zeros((nb, L, D + 2, D), np.float32)
    xpad[:, :, 1:D + 1] = xl
    for i in range(32):
        acc = np.zeros((nb, L, D), np.float32)
        for di in range(3):
            acc = acc + _conv1d3(xpad[:, :, 2 * i + di], Wl[di])
        h[:, :, i] = np.tanh(acc + bl)

    def S_prev(slot):
        out = np.zeros((nb, L, D), np.float32)
        if slot == 0:
            out[:, :] = xl[:, :, 63]
        else:
            out[:, 1:] = h[:, :-1, slot - 1]
        return out

    for lo, hi in ((32, 47), (48, 55), (56, 59), (60, 61), (62, 62)):
        for i in range(lo, hi + 1):
            acc = np.zeros((nb, L, D), np.float32)
            for di in range(3):
                acc = acc + _conv1d3(S_prev(2 * i - 64 + di), Wl[di])
            h[:, :, i] = np.tanh(acc + bl)

    dv = bl + _conv1d3(S_prev(62), Wl[0]) + _conv1d3(S_prev(63), Wl[1])
    H = np.zeros((nb, L, D), np.float32)
    deltas = []
    for _ in range(n_iter):
        Hp = np.zeros((nb, L, D), np.float32)
        Hp[:, 1:] = H[:, :-1]
        Hn = np.tanh(dv + _conv1d3(Hp, Wl[2]))
        deltas.append(float(np.abs(Hn - H).max()))
        H = Hn
    h[:, :, 63] = H
    return h, deltas


def _estimate_sweeps(x, Wn, bn):
    """Run the decomposition on one image, count sweeps until delta < 1e-8."""
    xs = x[:1].astype(np.float32)
    nits = []
    for l in range(2):
        xs_out, deltas = _numpy_layer(xs, Wn[l], bn[l], 40)
        nit = 40
        for k, d in enumerate(deltas):
            if d < 1e-8:
                nit = k + 1
                break
        nits.append(min(40, max(10, nit + 3)))
        xs = xs_out
    return nits


def _build_bass(bn, nits):
    import concourse.bass as bass
    import concourse.bacc as bacc
    import concourse.mybir as mybir
    import concourse.tile as tile

    f32 = mybir.dt.float32
    Tanh = mybir.ActivationFunctionType.Tanh

    nc = bacc.Bacc("TRN2", target_bir_lowering=False)
    xT = nc.dram_tensor("xT", [128, L, D], f32, kind="ExternalInput")
    bands = nc.dram_tensor("bands", [128, 6, 128], f32, kind="ExternalInput")
    outT = nc.dram_tensor("outT", [128, L, 63], f32, kind="ExternalOutput")
    row63 = nc.dram_tensor("row63", [128, L], f32, kind="ExternalOutput")
    s1dump = None
    if os.environ.get("BASS_DEBUG_DUMP"):
        s1dump = nc.dram_tensor("s1dump", [128, L + 1, 65], f32,
                                kind="ExternalOutput")

    with tile.TileContext(nc) as tc:
        with (
            tc.tile_pool(name="persist", bufs=1) as persist,
            tc.tile_pool(name="xpool", bufs=2) as xpool,
            tc.tile_pool(name="hpool", bufs=1) as hpool,
            tc.tile_pool(name="apool", bufs=2, space="PSUM") as apool,
            tc.tile_pool(name="rpool", bufs=2, space="PSUM") as rpool,
        ):
            bsb = persist.tile([128, 6, 128], f32)
            nc.sync.dma_start(out=bsb, in_=bands[:])

            def BD(l, di):
                return bsb[:, l * 3 + di, :]

            S = [persist.tile([128, L + 1, 65], f32, name=f"S{i}")
                 for i in range(2)]
            bias_t = [hpool.tile([128, 1], f32, name=f"bias{i}")
                      for i in range(2)]
            for i in range(2):
                nc.vector.memset(bias_t[i][:, :], float(bn[i]))
            H0 = hpool.tile([128, L + 1], f32)
            H1 = hpool.tile([128, L + 1], f32)
            nc.vector.memset(H0[:, :], 0.0)
            nc.vector.memset(H1[:, 0:1], 0.0)
            for l in range(2):
                nc.vector.memset(S[l][:, 0, :], 0.0)

            xt_tiles = {}

            def a_pass(l, blk):
                """Rows 0..31 for t in block; also stash slot0 copies."""
                Sl = S[l]
                if l == 0:
                    xt = xpool.tile([128, TB, D], f32)
                    nc.sync.dma_start(out=xt, in_=xT[:, blk * TB:(blk + 1) * TB, :])
                    xt_tiles[blk] = xt
                    # slot0[g=t] = x_t[row 63]
                    nc.vector.tensor_copy(
                        Sl[:, blk * TB:(blk + 1) * TB, 0], xt[:, :, 63])
                for half in range(TB // HB):
                    t0 = blk * TB + half * HB
                    pa = apool.tile([128, HB, 32], f32, name="pa", tag="acc")
                    for q in range(HB // 16):
                        lt = half * HB + q * 16
                        if l == 0:
                            src = xt_tiles[blk]
                            # taps: row 2i-1+di of x_t  (slot == row)
                            r0 = src[:, lt:lt + 16, 1:62:2]     # di=0, i=1..31
                            r1 = src[:, lt:lt + 16, 0:63:2]     # di=1, i=0..31
                            r2 = src[:, lt:lt + 16, 1:64:2]     # di=2, i=0..31
                        else:
                            # input = h1_t = S[0] group t+1, slot 1+row
                            g0 = t0 - half * HB * 0 + q * 16 + half * HB - q * 16  # noqa
                            gs = t0 + q * 16 + 1
                            src = S[0]
                            r0 = src[:, gs:gs + 16, 2:63:2]     # rows 1..61
                            r1 = src[:, gs:gs + 16, 1:64:2]     # rows 0..62
                            r2 = src[:, gs:gs + 16, 2:65:2]     # rows 1..63
                        o = pa[:, q * 16:(q + 1) * 16, :]
                        nc.tensor.matmul(o, BD(l, 1), r1, start=True, stop=False)
                        nc.tensor.matmul(o, BD(l, 2), r2, start=False, stop=False)
                        nc.tensor.matmul(o[:, :, 1:32], BD(l, 0), r0,
                                         start=False, stop=True)
                    nc.scalar.activation(
                        Sl[:, t0 + 1:t0 + HB + 1, 1:33], pa[:, :, :],
                        Tanh, bias=bias_t[l][:, :])

            def region_pass(l):
                """Cascade rows 32..62, region-major over the full sequence:
                each region reads only previous regions' rows at t-1."""
                Sl = S[l]
                for (ilo, ihi, NT) in ((32, 47, 32), (48, 55, 64),
                                       (56, 59, 128), (60, 61, 256),
                                       (62, 62, 256)):
                    n = ihi - ilo + 1
                    for t0 in range(0, L, NT):
                        pr = rpool.tile([128, NT, n], f32, name="pr", tag="reg")
                        for di in range(3):
                            s0 = 2 * ilo - 64 + di
                            rhs = Sl[:, t0:t0 + NT, s0:s0 + 2 * n - 1:2]
                            nc.tensor.matmul(pr[:, :, :], BD(l, di), rhs,
                                             start=(di == 0), stop=(di == 2))
                        nc.scalar.activation(
                            Sl[:, t0 + 1:t0 + NT + 1, 1 + ilo:2 + ihi],
                            pr[:, :, :], Tanh, bias=bias_t[l][:, :])

            def iterate(l):
                Sl = S[l]
                Hs = [H0, H1]
                for k in range(nits[l]):
                    pi = apool.tile([128, 256], f32, name="pi", tag="acc")
                    nc.tensor.matmul(pi, BD(l, 0), Sl[:, 0:L, 62],
                                     start=True, stop=False)
                    nc.tensor.matmul(pi, BD(l, 1), Sl[:, 0:L, 63],
                                     start=False, stop=False)
                    nc.tensor.matmul(pi, BD(l, 2), Hs[k % 2][:, 0:L],
                                     start=False, stop=True)
                    if k < nits[l] - 1:
                        nc.scalar.activation(Hs[(k + 1) % 2][:, 1:L + 1], pi,
                                             Tanh, bias=bias_t[l][:, :])
                    else:
                        nc.scalar.activation(Sl[:, 1:L + 1, 64], pi,
                                             Tanh, bias=bias_t[l][:, :])

            # ---- layer 1 ----
            for blk in range(NBLK):
                a_pass(0, blk)
            region_pass(0)
            iterate(0)
            # S2 slot0[g] = h1_g[63]
            nc.vector.tensor_copy(S[1][:, 0:L, 0], S[0][:, 1:L + 1, 64])
            # reset H for layer 2
            nc.vector.memset(H0[:, :], 0.0)

            # ---- layer 2 ----
            for blk in range(NBLK):
                a_pass(1, blk)
            region_pass(1)
            for blk in range(NBLK):
                nc.sync.dma_start(
                    out=outT[:, blk * TB:(blk + 1) * TB, :],
                    in_=S[1][:, blk * TB + 1:(blk + 1) * TB + 1, 1:64])
            iterate(1)
            nc.sync.dma_start(out=row63[:, :], in_=S[1][:, 1:L + 1, 64])
            if s1dump is not None:
                nc.sync.dma_start(out=s1dump[:, :, :], in_=S[0][:, :, :])

    nc.compile()
    return nc


def kernel(x, W, b):
    import sys
    if "/opt/trn_rl_repo" not in sys.path:
        sys.path.insert(0, "/opt/trn_rl_repo")
    from concourse.bass_utils import run_bass_kernel_spmd

    x = np.ascontiguousarray(np.asarray(x, np.float32))
    Wn = np.asarray(W, np.float32)[:, 0, 0]      # (2, 3, 3)
    bn = np.asarray(b, np.float32)               # (2,)

    nits = _estimate_sweeps(x, Wn, bn)
    nc = _build_bass(bn, nits)

    bands_np = _bands_tensor(Wn)
    in_maps = []
    for c in range(NCORES):
        xc = x[c * BS:(c + 1) * BS]                      # (2, L, D, D)
        xTc = np.ascontiguousarray(
            xc.transpose(0, 3, 1, 2).reshape(128, L, D))  # (img*j, t, row)
        in_maps.append({"xT": xTc, "bands": bands_np})

    res = run_bass_kernel_spmd(
        nc, in_maps, core_ids=list(range(NCORES)),
        trace=bool(int(os.environ.get("BASS_KERNEL_TRACE", "0"))))
    if os.environ.get("BASS_KERNEL_RESULT_PATH"):
        import pickle
        with open(os.environ["BASS_KERNEL_RESULT_PATH"], "wb") as f:
            pickle.dump({
                "exec_time_ns": res.exec_time_ns,
                "mean_exec_time_ns": res.mean_exec_time_ns,
                "trace": (res.instructions_and_trace or (None, None))[1],
                "profile_json": res.profile_json,
            }, f)

    out = np.empty((B, L, D, D), np.float32)
    for c in range(NCORES):
        r = res.results[c]
        main = r["outT"].reshape(BS, D, L, 63)           # (img, j, t, row)
        r63 = r["row63"].reshape(BS, D, L)               # (img, j, t)
        out[c * BS:(c + 1) * BS, :, 0:63, :] = main.transpose(0, 2, 3, 1)
        out[c * BS:(c + 1) * BS, :, 63, :] = r63.transpose(0, 2, 1)
    return out

